# revision 2
# baseline (speedup 1.0000x reference)
"""Trainium2 Bass kernel for nn_CATCallerEncoderLayer (dynamic-conv encoder layer).

Reference computation (T=1024, B=16, C=512, H=8, K=31, P=15):
  h  = x @ w1 + b1; a, g = split(h); xg = a * sigmoid(g)
  w  = softmax((xg @ wl_w + wl_b).reshape(T,B,H,K), axis=-1)
  out[t,b,h*64+r] = sum_k w[t,b,h,k] * xg_pad[t+k-15, b, h*64+r]
  return out @ w2 + b2

Sharding: data-parallel over B across 8 cores (2 batches/core). Host supplies
x pre-transposed to feature-major [b, C, T] and wl_w zero-padded to 256 cols.

Per core:
  mm1 (f32r, lhsT = xT slices)  -> h1 token-major PSUM -> GLU -> xg [t,C] SBUF
  PE-transpose xg (f32)         -> mm_dyn (f32r, N=256) -> w248 [t,248] PSUM
  exp / group-sum / recip       -> wsoft -> cast bf16
  M-form shear-write (bf16) to zero-filled DRAM staging:
      stage[(b,h,i)-block row t_loc, col t_loc+k] = wsoft[t, h, k]  (98-blocks)
  readback [98, 8*128] -> 8 PE-transposes (bf16) -> band.T tiles [s_pad, t]
  conv: psum[c64x2, t] += halo_bf16[:, h].T @ band_h   (banded matmul)
  mm2 (f32r, lhsT = conv feature-major tiles) -> out [t, C] -> DRAM
"""
import sys

sys.path.insert(0, "/opt/trn_rl_repo")

import numpy as np

T, B, C = 1024, 16, 512
H, KT, PAD = 8, 31, 15
HK = H * KT          # 248
HKP = 256            # wl_w padded cols
NCORES = 8
BPC = B // NCORES    # 2
BT = 98              # conv time-block (s_pad = t + k <= 127)
NBLK = (T + BT - 1) // BT  # 11
NTC = T // 128       # 8
NCC = C // 128       # 4
SBLK = 98 * 128      # staging elements per (b,h,i) block

_cache = {}


def _split_sync_waits(nc, mybir, max_waits=1):
    """This walrus build rejects instructions carrying >1 sync-wait command.
    Hoist extra waits onto same-engine NOPs inserted just before."""
    cnt = 0
    for f in nc.m.functions:
        for bb in f.blocks:
            new = []
            for inst in bb.instructions:
                si = inst.sync_info
                if si is not None and si.on_wait and len(si.on_wait) > max_waits:
                    waits = list(si.on_wait)
                    for w in waits[:-max_waits]:
                        cnt += 1
                        new.append(
                            mybir.InstNoOp(
                                name=f"I-ws{cnt}",
                                engine=inst.engine,
                                sync_info=mybir.SyncInfo(on_wait=[w], on_update=[]),
                            )
                        )
                    inst.sync_info = mybir.SyncInfo(
                        on_wait=waits[-max_waits:], on_update=list(si.on_update or [])
                    )
                new.append(inst)
            bb.instructions = new
    return cnt


def _build(has_b1, has_wlb, has_b2, reps=1, reps_a=None, reps_b=None):
    import bass_rust
    import concourse.bass as bass
    import concourse.tile as tile
    from concourse import mybir

    f32 = mybir.dt.float32
    f32r = mybir.dt.float32r
    bf16 = mybir.dt.bfloat16
    AF = mybir.ActivationFunctionType
    AX = mybir.AxisListType

    def r(ap):
        return ap.bitcast(f32r)

    nc = bass.Bass("TRN2", debug=False)

    xt_d = nc.dram_tensor("xt", (BPC, C, T), f32, kind="ExternalInput").ap()
    w1_d = nc.dram_tensor("w1", (C, 2 * C), f32, kind="ExternalInput").ap()
    wlw_d = nc.dram_tensor("wl_w", (C, HKP), f32, kind="ExternalInput").ap()
    w2_d = nc.dram_tensor("w2", (C, C), f32, kind="ExternalInput").ap()
    b1_d = nc.dram_tensor("b1", (2 * C,), f32, kind="ExternalInput").ap()
    wlb_d = nc.dram_tensor("wl_b", (HK,), f32, kind="ExternalInput").ap()
    b2_d = nc.dram_tensor("b2", (C,), f32, kind="ExternalInput").ap()
    eye_d = nc.dram_tensor("eye", (128, 128), f32, kind="ExternalInput").ap()
    out_d = nc.dram_tensor("out", (T, BPC, C), f32, kind="ExternalOutput").ap()
    # bf16 M-form band staging: per (b,h,i) block of [98, 128]
    n_sblk = BPC * H * NBLK
    stage = nc.dram_tensor("stage", (n_sblk * 98, 128), bf16).ap()
    stage_f = stage[:].flatten()

    def sbase(b, h, i):
        return (((b * H) + h) * NBLK + i) * SBLK

    with tile.TileContext(nc) as tc:
        with (
            tc.tile_pool(name="consts", bufs=1) as cpool,
            tc.tile_pool(name="xt", bufs=BPC * NCC) as xtpool,
            tc.tile_pool(name="xg", bufs=BPC * NTC) as xgpool,
            tc.tile_pool(name="xgb", bufs=BPC * NTC) as xgbpool,
            tc.tile_pool(name="work", bufs=3) as wpool,
            tc.tile_pool(name="xgf", bufs=6) as xgfpool,
            tc.tile_pool(name="soft", bufs=3) as spool,
            tc.tile_pool(name="mread", bufs=3) as mpool,
            tc.tile_pool(name="band", bufs=10) as bpool,
            tc.tile_pool(name="halo", bufs=3) as hpool,
            tc.tile_pool(name="outp", bufs=3) as opool,
        ):
            # ---- constants ----
            w1_t, wlw_t, w2_t = [], [], []
            for cc in range(NCC):
                tw1 = cpool.tile([128, 2 * C], f32r, tag=f"w1_{cc}")
                nc.gpsimd.dma_start(tw1[:], w1_d[cc * 128:(cc + 1) * 128, :])
                w1_t.append(tw1)
                twl = cpool.tile([128, HKP], f32r, tag=f"wlw_{cc}")
                nc.gpsimd.dma_start(twl[:], wlw_d[cc * 128:(cc + 1) * 128, :])
                wlw_t.append(twl)
                tw2 = cpool.tile([128, C], f32r, tag=f"w2_{cc}")
                nc.gpsimd.dma_start(tw2[:], w2_d[cc * 128:(cc + 1) * 128, :])
                w2_t.append(tw2)
            eye = cpool.tile([128, 128], f32, tag="eye")
            nc.sync.dma_start(eye[:], eye_d[:])
            eyeb = cpool.tile([128, 128], bf16, tag="eyeb")
            nc.vector.tensor_copy(eyeb[:], eye[:])
            if has_b1:
                b1a = cpool.tile([128, C], f32, tag="b1a")
                nc.sync.dma_start(b1a[:], b1_d[None, 0:C].to_broadcast((128, C)))
                b1g = cpool.tile([128, C], f32, tag="b1g")
                nc.sync.dma_start(b1g[:], b1_d[None, C:2 * C].to_broadcast((128, C)))
            if has_wlb:
                wlb = cpool.tile([128, HK], f32, tag="wlb")
                nc.sync.dma_start(wlb[:], wlb_d[None, :].to_broadcast((128, HK)))
            if has_b2:
                b2t = cpool.tile([128, C], f32, tag="b2t")
                nc.sync.dma_start(b2t[:], b2_d[None, :].to_broadcast((128, C)))

            # zero-fill staging (garbage-free band tiles; persists across reps)
            zt = cpool.tile([128, 2048], bf16, tag="zt")
            nc.vector.memset(zt[:], 0.0)
            nrows = n_sblk * 98
            zrows = 2048
            pos = 0
            while pos < nrows:
                n = min(zrows, nrows - pos)
                nc.scalar.dma_start(stage[pos:pos + n, :], zt[:, :n])
                pos += n

            xt_t = {}
            for b in range(BPC):
                for cc in range(NCC):
                    tx = xtpool.tile([128, T], f32r, tag="xt")
                    nc.gpsimd.dma_start(tx[:], xt_d[b, cc * 128:(cc + 1) * 128, :])
                    xt_t[(b, cc)] = tx

            if reps_a is None:
                reps_a = reps
            if reps_b is None:
                reps_b = reps
            xg_t = {}
            xgb_t = {}
            for rep in range(reps_a):

                # ================= Phase A =================
                with tc.tile_pool(name=f"psumA{rep}", bufs=2, space="PSUM") as psA:
                    for b in range(BPC):
                        for tch in range(NTC):
                            t0 = tch * 128
                            h1a = psA.tile([128, C], f32, tag="h1a")
                            h1g = psA.tile([128, C], f32, tag="h1g")
                            for cc in range(NCC):
                                lhsT = xt_t[(b, cc)][:, t0:t0 + 128]
                                nc.tensor.matmul(h1a[:], lhsT, w1_t[cc][:, 0:C],
                                                 start=(cc == 0), stop=(cc == NCC - 1))
                            for cc in range(NCC):
                                lhsT = xt_t[(b, cc)][:, t0:t0 + 128]
                                nc.tensor.matmul(h1g[:], lhsT, w1_t[cc][:, C:2 * C],
                                                 start=(cc == 0), stop=(cc == NCC - 1))
                            # GLU: xg = (a + b1a) * sigmoid(g + b1g)
                            sg = wpool.tile([128, C], f32, tag="sg")
                            if has_b1:
                                gb = wpool.tile([128, C], f32, tag="gb")
                                nc.vector.tensor_add(gb[:], h1g[:], b1g[:])
                                nc.scalar.activation(sg[:], gb[:], AF.Sigmoid)
                            else:
                                nc.scalar.activation(sg[:], h1g[:], AF.Sigmoid)
                            xg = xgpool.tile([128, C], f32, tag="xg")
                            if has_b1:
                                ab = wpool.tile([128, C], f32, tag="ab")
                                nc.vector.tensor_add(ab[:], h1a[:], b1a[:])
                                nc.vector.tensor_mul(xg[:], ab[:], sg[:])
                            else:
                                nc.vector.tensor_mul(xg[:], h1a[:], sg[:])
                            xg_t[(b, tch)] = xg
                            xgb = xgbpool.tile([128, C], bf16, tag="xgb")
                            nc.vector.tensor_copy(xgb[:], xg[:])
                            xgb_t[(b, tch)] = xgb

                            # transpose xg -> feature-major chunks, then mm_dyn
                            w248 = psA.tile([128, HKP], f32, tag="w248")
                            for cc in range(NCC):
                                tp = psA.tile([128, 128], f32, tag="tp")
                                nc.tensor.transpose(tp[:], xg[:, cc * 128:(cc + 1) * 128],
                                                    eye[:])
                                xgf = xgfpool.tile([128, 128], f32r, tag="xgf")
                                nc.scalar.copy(xgf[:], tp[:])
                                nc.tensor.matmul(w248[:], xgf[:], wlw_t[cc][:],
                                                 start=(cc == 0), stop=(cc == NCC - 1))
                            # softmax over taps (values tiny: skip max-subtract)
                            we = spool.tile([128, HK], f32, tag="we")
                            if has_wlb:
                                wb = spool.tile([128, HK], f32, tag="wb")
                                nc.vector.tensor_add(wb[:], w248[:, 0:HK], wlb[:])
                                nc.scalar.activation(we[:], wb[:], AF.Exp)
                            else:
                                nc.scalar.activation(we[:], w248[:, 0:HK], AF.Exp)
                            sums = spool.tile([128, H], f32, tag="sums")
                            we3 = we[:].rearrange("t (h k) -> t h k", k=KT)
                            nc.vector.reduce_sum(sums[:], we3, axis=AX.X)
                            rec = spool.tile([128, H], f32, tag="rec")
                            nc.vector.reciprocal(rec[:], sums[:])
                            wsb = spool.tile([128, HK], bf16, tag="wsb")
                            ws3 = wsb[:].rearrange("t (h k) -> t h k", k=KT)
                            rec3 = rec[:, :, None].to_broadcast((128, H, KT))
                            nc.vector.tensor_mul(ws3, we3, rec3)
                            # M-form shear-write to staging: all 8 heads per DMA,
                            # split by overlapped 98-blocks. Element (t,h,k) goes
                            # to sbase(b,h,i) + t_loc*129 + k.
                            g0, g1 = t0, t0 + 128
                            i = g0 // BT
                            while i < NBLK and i * BT < g1:
                                r0 = max(g0, i * BT)
                                r1 = min(g1, i * BT + BT)
                                nr = r1 - r0
                                tl0 = r0 - i * BT
                                src = wsb[r0 - g0:r1 - g0, :].rearrange(
                                    "t (h k) -> t h k", k=KT)
                                dst = stage_f[:1].copy()
                                dst.ap = bass_rust.VecI64Pair(
                                    [[129, nr], [NBLK * SBLK, H], [1, KT]])
                                dst.offset = sbase(b, 0, i) + tl0 * 129
                                nc.sync.dma_start(dst, src)
                                i += 1

            for rep in range(reps_b):
                # ================= Phase B =================
                with tc.tile_pool(name=f"psumB{rep}", bufs=2, space="PSUM") as psB:
                    for b in range(BPC):
                        for i in range(NBLK):
                            t0 = i * BT
                            blk = min(BT, T - t0)
                            # halo tile: bf16 xg rows [t0-15, t0-15+128)
                            halo = hpool.tile([128, C], bf16, tag="halo")
                            lo = t0 - PAD
                            hi = lo + 128
                            vlo, vhi = max(lo, 0), min(hi, T)
                            if vlo > lo or vhi < hi:
                                nc.vector.memset(halo[:], 0.0)
                            rr = vlo
                            while rr < vhi:
                                src = xgb_t[(b, rr // 128)]
                                n = min(vhi - rr, 128 - (rr % 128))
                                nc.sync.dma_start(
                                    halo[rr - lo:rr - lo + n, :],
                                    src[rr % 128:rr % 128 + n, :],
                                )
                                rr += n
                            # readback M-form [98, 8*128] in one DMA
                            mt = mpool.tile([98, H * 128], f32, tag="mt")
                            mt3 = mt[:].rearrange("t (h s) -> t h s", s=128)
                            rd = stage_f[:1].copy()
                            rd.ap = bass_rust.VecI64Pair(
                                [[128, 98], [NBLK * SBLK, H], [1, 128]])
                            rd.offset = sbase(b, 0, i)
                            nc.gpsimd.dma_start(mt3, rd)
                            # per head: PE-transpose -> band.T, then conv matmuls
                            cp = psB.tile([128, NCC * blk], f32, tag="cp")
                            for j in range(NCC):
                                for jj in range(2):
                                    h = 2 * j + jj
                                    btp = psB.tile([128, 98], f32, tag="btp")
                                    nc.tensor.transpose(
                                        btp[:], mt[:, h * 128:(h + 1) * 128],
                                        eye[:98, :98])
                                    band = bpool.tile([128, 98], bf16, tag="band")
                                    if (j + jj) % 2 == 0:
                                        nc.vector.tensor_copy(band[:], btp[:])
                                    else:
                                        nc.scalar.copy(band[:], btp[:])
                                    nc.tensor.matmul(
                                        cp[jj * 64:(jj + 1) * 64, j * blk:(j + 1) * blk],
                                        halo[:, h * 64:(h + 1) * 64],
                                        band[:, :blk],
                                        start=True, stop=True,
                                        tile_position=(0, 64 * jj),
                                    )
                            convT = opool.tile([128, NCC * blk], f32r, tag="convT")
                            nc.vector.tensor_copy(convT[:], cp[:])
                            # mm2
                            op = psB.tile([128, C], f32, tag="op")
                            for j in range(NCC):
                                nc.tensor.matmul(op[:blk, :],
                                                 convT[:, j * blk:(j + 1) * blk],
                                                 w2_t[j][:],
                                                 start=(j == 0), stop=(j == NCC - 1))
                            os = opool.tile([128, C], f32, tag="os")
                            if has_b2:
                                nc.vector.tensor_add(os[:blk, :], op[:blk, :],
                                                     b2t[:blk, :])
                            else:
                                nc.scalar.copy(os[:blk, :], op[:blk, :])
                            nc.sync.dma_start(out_d[t0:t0 + blk, b, :], os[:blk, :])

    _split_sync_waits(nc, mybir)
    return nc


def _make_in_maps(inputs):
    x = np.asarray(inputs["x"], np.float32)
    w1 = np.asarray(inputs["w1"], np.float32)
    b1 = np.asarray(inputs["b1"], np.float32)
    wl_w = np.asarray(inputs["wl_w"], np.float32)
    wl_b = np.asarray(inputs["wl_b"], np.float32)
    w2 = np.asarray(inputs["w2"], np.float32)
    b2 = np.asarray(inputs["b2"], np.float32)

    eye = np.eye(128, dtype=np.float32)
    wlw_pad = np.zeros((C, HKP), np.float32)
    wlw_pad[:, :HK] = wl_w

    in_maps = []
    for c in range(NCORES):
        xs = x[:, c * BPC:(c + 1) * BPC, :]  # (T, BPC, C)
        xt = np.ascontiguousarray(xs.transpose(1, 2, 0))  # (BPC, C, T)
        in_maps.append({
            "xt": xt, "w1": w1, "wl_w": wlw_pad, "w2": w2,
            "b1": b1, "wl_b": wl_b, "b2": b2,
            "eye": eye,
        })
    return in_maps


def kernel(x, w1, b1, wl_w, wl_b, w2, b2):
    from concourse.bass_utils import run_bass_kernel_spmd

    b1 = np.asarray(b1, np.float32)
    wl_b = np.asarray(wl_b, np.float32)
    b2 = np.asarray(b2, np.float32)

    has_b1 = bool(np.any(b1))
    has_wlb = bool(np.any(wl_b))
    has_b2 = bool(np.any(b2))

    key = (has_b1, has_wlb, has_b2)
    if key not in _cache:
        _cache[key] = _build(*key)
    nc = _cache[key]

    in_maps = _make_in_maps({
        "x": x, "w1": w1, "b1": b1, "wl_w": wl_w,
        "wl_b": wl_b, "w2": w2, "b2": b2,
    })

    res = run_bass_kernel_spmd(nc, in_maps, core_ids=list(range(NCORES)))
    out = np.empty((T, B, C), np.float32)
    for c in range(NCORES):
        out[:, c * BPC:(c + 1) * BPC, :] = res.results[c]["out"]
    return out



# revision 19
# speedup vs baseline: 1.3661x; 1.3661x over previous
"""Trainium2 Bass kernel for nn_CATCallerEncoderLayer (dynamic-conv encoder layer).

Reference computation (T=1024, B=16, C=512, H=8, K=31, P=15):
  h  = x @ w1 + b1; a, g = split(h); xg = a * sigmoid(g)
  w  = softmax((xg @ wl_w + wl_b).reshape(T,B,H,K), axis=-1)
  out[t,b,h*64+r] = sum_k w[t,b,h,k] * xg_pad[t+k-15, b, h*64+r]
  return out @ w2 + b2

Sharding: data-parallel over batch B across 8 cores (2 batches/core). Host
supplies x pre-transposed to feature-major [b, C, T] in bf16 and the weights
in bf16 (wl_w zero-padded to 256 cols). All matmuls run in bf16.

Per core (b = batch slot, 2 per core):
  Phase A per 128-token tile:
    mm1 (bf16, lhsT = xT slices) -> h1a/h1g PSUM -> sigmoid (ACT) ->
    xgb = a*sig(g) bf16 (DVE) -> DRAM xg_pad (halo source)
    xgT via 4 SBUF->SBUF DMA-xbar transposes -> mm_dyn (bf16, N=256)
    exp (ACT) -> group-sum/recip (DVE) -> wsoft bf16
    M-form shear-write (bf16) to zero-filled DRAM staging:
      stage[(b,i,h)-block row t_loc, col t_loc+k] = wsoft[t, h, k]
  Phase B per 98-token conv block:
    band8 [s, (h,t)] via ONE DMA-xbar transpose of the [784, 128] stage block
    halo [128, 512] read from DRAM xg_pad (token rows t0-15 .. t0+113)
    conv psum[c64x2, 4*blk] += halo[:, h].T @ band8[:, h]  (banded matmul)
    convT[b] feature-major accumulator [128, 4, 1024] bf16
  mm2 per 128-token tile (bf16, lhsT = convT slices) -> out bf16 -> DRAM
Host casts the bf16 output back to f32.
"""
import sys

sys.path.insert(0, "/opt/trn_rl_repo")

import numpy as np
import ml_dtypes

T, B, C = 1024, 16, 512
H, KT, PAD = 8, 31, 15
HK = H * KT          # 248
HKP = 256            # wl_w padded cols
NCORES = 8
BPC = B // NCORES    # 2
BT = 98              # conv time-block (s_pad = t + k <= 127)
NBLK = (T + BT - 1) // BT  # 11
NTC = T // 128       # 8
NCC = C // 128       # 4
SBLK = BT * 128      # staging elements per (b,i,h) block = 12544
XGPR = BT * (NBLK - 1) + 128   # xg_pad rows per batch = 1108

_cache = {}
_DEBUG_OUTPUTS = False
_BARRIER_AFTER_ZF = False
# SBUF->SBUF DMA-xbar transposes corrupt concurrent DMA traffic on this HW
# (observed as nondeterministic garbage under multi-core HBM contention), so
# the token->feature xg transpose runs on the PE instead. DRAM->SBUF xbar
# transposes (band8 readback) are unaffected.
_PE_XGT = True


def _split_sync_waits(nc, mybir, max_waits=1):
    """This walrus build rejects instructions carrying >1 sync-wait command.
    Hoist extra waits onto same-engine NOPs inserted just before."""
    cnt = 0
    for f in nc.m.functions:
        for bb in f.blocks:
            new = []
            for inst in bb.instructions:
                si = inst.sync_info
                if si is not None and si.on_wait and len(si.on_wait) > max_waits:
                    waits = list(si.on_wait)
                    for w in waits[:-max_waits]:
                        cnt += 1
                        new.append(
                            mybir.InstNoOp(
                                name=f"I-ws{cnt}",
                                engine=inst.engine,
                                sync_info=mybir.SyncInfo(on_wait=[w], on_update=[]),
                            )
                        )
                    inst.sync_info = mybir.SyncInfo(
                        on_wait=waits[-max_waits:], on_update=list(si.on_update or [])
                    )
                new.append(inst)
            bb.instructions = new
    return cnt


def _build(has_b1, has_wlb, has_b2):
    import bass_rust
    import concourse.bass as bass
    import concourse.tile as tile
    from concourse import mybir

    f32 = mybir.dt.float32
    bf16 = mybir.dt.bfloat16
    AF = mybir.ActivationFunctionType
    AX = mybir.AxisListType

    nc = bass.Bass("TRN2", debug=False)

    xt_d = nc.dram_tensor("xt", (BPC, C, T), bf16, kind="ExternalInput").ap()
    w1_d = nc.dram_tensor("w1", (C, 2 * C), bf16, kind="ExternalInput").ap()
    wlw_d = nc.dram_tensor("wl_w", (C, HKP), bf16, kind="ExternalInput").ap()
    w2_d = nc.dram_tensor("w2", (C, C), bf16, kind="ExternalInput").ap()
    eye_d = nc.dram_tensor("eye", (128, 128), bf16, kind="ExternalInput").ap()
    if has_b1:
        b1_d = nc.dram_tensor("b1", (2 * C,), f32, kind="ExternalInput").ap()
    if has_wlb:
        wlb_d = nc.dram_tensor("wl_b", (HK,), f32, kind="ExternalInput").ap()
    if has_b2:
        b2_d = nc.dram_tensor("b2", (C,), f32, kind="ExternalInput").ap()
    out_d = nc.dram_tensor("out", (T, BPC, C), bf16, kind="ExternalOutput").ap()
    dbg = "ExternalOutput" if _DEBUG_OUTPUTS else "Internal"
    # bf16 M-form band staging: per (b,i,h) block of [98, 128]
    n_srow = BPC * NBLK * H * BT        # 17248 rows
    stage = nc.dram_tensor("stage", (n_srow, 128), bf16, kind=dbg).ap()
    stage_f = stage[:].flatten()
    # zero-padded token-major xg for conv halos: row r = token r-15
    xgp_d = nc.dram_tensor("xgp", (BPC, XGPR, C), bf16, kind=dbg).ap()

    def sbase(b, i, h=0):
        return ((b * NBLK + i) * H + h) * SBLK

    with tile.TileContext(nc) as tc:
        with (
            tc.tile_pool(name="consts", bufs=1) as cpool,
            tc.tile_pool(name="xt", bufs=BPC * NCC) as xtpool,
            tc.tile_pool(name="convT", bufs=BPC) as ctpool,
            tc.tile_pool(name="xgb", bufs=3) as xgbpool,
            tc.tile_pool(name="xgT", bufs=3) as xgtpool,
            tc.tile_pool(name="sg", bufs=3) as sgpool,
            tc.tile_pool(name="soft", bufs=3) as spool,
            tc.tile_pool(name="band", bufs=4) as bpool,
            tc.tile_pool(name="halo", bufs=4) as hpool,
            tc.tile_pool(name="outp", bufs=3) as opool,
            tc.tile_pool(name="psA", bufs=2, space="PSUM") as psA,
            tc.tile_pool(name="psW", bufs=1, space="PSUM") as psW,
            tc.tile_pool(name="psB", bufs=1, space="PSUM") as psB,
            tc.tile_pool(name="psO", bufs=1, space="PSUM") as psO,
            tc.tile_pool(name="psT", bufs=1, space="PSUM") as psT,
        ):
            # ---- constants ----
            w1_t, wlw_t, w2_t = [], [], []
            for cc in range(NCC):
                tw1 = cpool.tile([128, 2 * C], bf16, tag=f"w1_{cc}")
                nc.gpsimd.dma_start(tw1[:], w1_d[cc * 128:(cc + 1) * 128, :])
                w1_t.append(tw1)
                twl = cpool.tile([128, HKP], bf16, tag=f"wlw_{cc}")
                nc.gpsimd.dma_start(twl[:], wlw_d[cc * 128:(cc + 1) * 128, :])
                wlw_t.append(twl)
                tw2 = cpool.tile([128, C], bf16, tag=f"w2_{cc}")
                nc.gpsimd.dma_start(tw2[:], w2_d[cc * 128:(cc + 1) * 128, :])
                w2_t.append(tw2)
            if has_b1:
                b1a = cpool.tile([128, C], f32, tag="b1a")
                nc.gpsimd.dma_start(b1a[:], b1_d[None, 0:C].to_broadcast((128, C)))
                b1g = cpool.tile([128, C], f32, tag="b1g")
                nc.gpsimd.dma_start(b1g[:], b1_d[None, C:2 * C].to_broadcast((128, C)))
            if has_wlb:
                wlb = cpool.tile([128, HK], f32, tag="wlb")
                nc.gpsimd.dma_start(wlb[:], wlb_d[None, :].to_broadcast((128, HK)))
            if has_b2:
                b2t = cpool.tile([128, C], f32, tag="b2t")
                nc.gpsimd.dma_start(b2t[:], b2_d[None, :].to_broadcast((128, C)))

            # zero-fill staging + xg_pad edges (garbage-free band/halo reads)
            zt = cpool.tile([128, 2048], bf16, tag="zt")
            nc.vector.memset(zt[:], 0.0)
            pos = 0
            while pos < n_srow:
                n = min(2048, n_srow - pos)
                nc.scalar.dma_start(stage[pos:pos + n, :], zt[:, :n])
                pos += n
            for b in range(BPC):
                nc.scalar.dma_start(xgp_d[b, 0:PAD, :], zt[:, :PAD * NCC])
                ztail = XGPR - (T + PAD)   # 69
                nc.scalar.dma_start(xgp_d[b, T + PAD:XGPR, :], zt[:, :ztail * NCC])

            xt_t = {}
            for b in range(BPC):
                for cc in range(NCC):
                    tx = xtpool.tile([128, T], bf16, tag="xt")
                    nc.gpsimd.dma_start(tx[:], xt_d[b, cc * 128:(cc + 1) * 128, :])
                    xt_t[(b, cc)] = tx

            if _BARRIER_AFTER_ZF:
                tc.strict_bb_all_engine_barrier()

            eyeb = cpool.tile([128, 128], bf16, tag="eyeb")
            nc.gpsimd.dma_start(eyeb[:], eye_d[:])

            convT = {}
            for b in range(BPC):
                ct = ctpool.tile([128, NCC, T], bf16, tag="convT")
                convT[b] = ct

            def phase_a(b, tch):
                t0 = tch * 128
                h1a = psA.tile([128, C], f32, tag="h1a")
                h1g = psA.tile([128, C], f32, tag="h1g")
                for cc in range(NCC):
                    lhsT = xt_t[(b, cc)][:, t0:t0 + 128]
                    nc.tensor.matmul(h1a[:], lhsT, w1_t[cc][:, 0:C],
                                     start=(cc == 0), stop=(cc == NCC - 1))
                for cc in range(NCC):
                    lhsT = xt_t[(b, cc)][:, t0:t0 + 128]
                    nc.tensor.matmul(h1g[:], lhsT, w1_t[cc][:, C:2 * C],
                                     start=(cc == 0), stop=(cc == NCC - 1))
                # GLU via tanh (keeps ACT on the exp_and_others table set):
                #   2*xg = (1 + tanh((g+b1g)/2)) * (a + b1a)
                # The factor 2 is compensated by host-side 0.5x pre-scaling
                # of wl_w and w2.
                sg = sgpool.tile([128, C], bf16, tag="sg")
                if has_b1:
                    gb = sgpool.tile([128, C], f32, tag="gb")
                    nc.vector.tensor_add(gb[:], h1g[:], b1g[:])
                    nc.scalar.activation(sg[:], gb[:], AF.Tanh, scale=0.5)
                else:
                    nc.scalar.activation(sg[:], h1g[:], AF.Tanh, scale=0.5)
                xgb = xgbpool.tile([128, C], bf16, tag="xgb")
                AL = mybir.AluOpType
                if has_b1:
                    ab = sgpool.tile([128, C], f32, tag="ab")
                    nc.vector.tensor_add(ab[:], h1a[:], b1a[:])
                    nc.vector.scalar_tensor_tensor(
                        xgb[:], sg[:], 1.0, ab[:], AL.add, AL.mult)
                else:
                    nc.vector.scalar_tensor_tensor(
                        xgb[:], sg[:], 1.0, h1a[:], AL.add, AL.mult)
                # halo source in DRAM (row r = token r-15)
                nc.sync.dma_start(xgp_d[b, PAD + t0:PAD + t0 + 128, :], xgb[:])
                # feature-major xg
                xgT = xgtpool.tile([128, C], bf16, tag="xgT")
                if _PE_XGT:
                    tp = psT.tile([128, C], bf16, tag="tp")
                    for j in range(NCC):
                        nc.tensor.transpose(tp[:, j * 128:(j + 1) * 128],
                                            xgb[:, j * 128:(j + 1) * 128],
                                            eyeb[:])
                    nc.vector.tensor_copy(xgT[:], tp[:])
                else:
                    for j in range(NCC):
                        nc.scalar.dma_start_transpose(
                            xgT[:, j * 128:(j + 1) * 128],
                            xgb[:, j * 128:(j + 1) * 128])
                # mm_dyn
                w248 = psW.tile([128, HKP], f32, tag="w248")
                for j in range(NCC):
                    nc.tensor.matmul(w248[:], xgT[:, j * 128:(j + 1) * 128],
                                     wlw_t[j][:],
                                     start=(j == 0), stop=(j == NCC - 1))
                # softmax over taps (values tiny: skip max-subtract)
                we = spool.tile([128, HK], bf16, tag="we")
                if has_wlb:
                    wb = spool.tile([128, HK], f32, tag="wb")
                    nc.vector.tensor_add(wb[:], w248[:, 0:HK], wlb[:])
                    nc.scalar.activation(we[:], wb[:], AF.Exp)
                else:
                    nc.scalar.activation(we[:], w248[:, 0:HK], AF.Exp)
                sums = spool.tile([128, H], f32, tag="sums")
                we3 = we[:].rearrange("t (h k) -> t h k", k=KT)
                nc.vector.reduce_sum(sums[:], we3, axis=AX.X)
                rec = spool.tile([128, H], f32, tag="rec")
                nc.vector.reciprocal(rec[:], sums[:])
                wsb = spool.tile([128, HK], bf16, tag="wsb")
                ws3 = wsb[:].rearrange("t (h k) -> t h k", k=KT)
                rec3 = rec[:, :, None].to_broadcast((128, H, KT))
                nc.vector.tensor_mul(ws3, we3, rec3)
                # M-form shear-write to staging: all 8 heads per DMA,
                # split by overlapped 98-blocks. Element (t,h,k) goes
                # to sbase(b,i,h) + t_loc*129 + k.
                g0, g1 = t0, t0 + 128
                i = g0 // BT
                while i < NBLK and i * BT < g1:
                    r0 = max(g0, i * BT)
                    r1 = min(g1, i * BT + BT)
                    nr = r1 - r0
                    tl0 = r0 - i * BT
                    src = wsb[r0 - g0:r1 - g0, :].rearrange(
                        "t (h k) -> t h k", k=KT)
                    dst = stage_f[:1].copy()
                    dst.ap = bass_rust.VecI64Pair(
                        [[129, nr], [SBLK, H], [1, KT]])
                    dst.offset = sbase(b, i) + tl0 * 129
                    nc.sync.dma_start(dst, src)
                    i += 1

            def conv_block(b, i):
                t0 = i * BT
                blk = min(BT, T - t0)
                # band8[s, h*98+t] via one DMA-xbar transpose of stage block
                band8 = bpool.tile([128, H * BT], bf16, tag="band8")
                r0 = (b * NBLK + i) * H * BT
                nc.sync.dma_start_transpose(band8[:], stage[r0:r0 + H * BT, :])
                # halo: token rows [t0-15, t0+113) = xgp rows [t0, t0+128)
                halo = hpool.tile([128, C], bf16, tag="halo")
                nc.gpsimd.dma_start(halo[:], xgp_d[b, t0:t0 + 128, :])
                # banded conv matmuls: 2 heads per psum tile via col groups
                cp = psB.tile([128, NCC * blk], f32, tag="cp")
                for j in range(NCC):
                    for jj in range(2):
                        h = 2 * j + jj
                        nc.tensor.matmul(
                            cp[jj * 64:(jj + 1) * 64, j * blk:(j + 1) * blk],
                            halo[:, h * 64:(h + 1) * 64],
                            band8[:, h * BT:h * BT + blk],
                            start=True, stop=True,
                            tile_position=(0, 64 * jj),
                        )
                # feature-major conv accumulator (bf16) for mm2 lhsT
                cp3 = cp[:].rearrange("p (j t) -> p j t", j=NCC)
                nc.vector.tensor_copy(convT[b][:, :, t0:t0 + blk], cp3)

            def mm2_tile(b, tch):
                t0 = tch * 128
                op = psO.tile([128, C], f32, tag="op")
                for j in range(NCC):
                    nc.tensor.matmul(op[:], convT[b][:, j, t0:t0 + 128],
                                     w2_t[j][:],
                                     start=(j == 0), stop=(j == NCC - 1))
                os = opool.tile([128, C], bf16, tag="os")
                if has_b2:
                    nc.vector.tensor_add(os[:], op[:], b2t[:])
                else:
                    nc.scalar.copy(os[:], op[:])
                nc.sync.dma_start(out_d[t0:t0 + 128, b, :], os[:])

            for b in range(BPC):
                for tch in range(NTC):
                    phase_a(b, tch)
                mm2_done = 0
                for i in range(NBLK):
                    conv_block(b, i)
                    # interleave mm2 for fully-covered token tiles
                    while (mm2_done + 1) * 128 <= (i + 1) * BT and mm2_done < NTC:
                        mm2_tile(b, mm2_done)
                        mm2_done += 1
                while mm2_done < NTC:
                    mm2_tile(b, mm2_done)
                    mm2_done += 1

    _split_sync_waits(nc, mybir)
    return nc


def _make_in_maps(inputs):
    x = np.asarray(inputs["x"], np.float32)
    w1 = np.asarray(inputs["w1"], np.float32)
    b1 = np.asarray(inputs["b1"], np.float32)
    wl_w = np.asarray(inputs["wl_w"], np.float32)
    wl_b = np.asarray(inputs["wl_b"], np.float32)
    w2 = np.asarray(inputs["w2"], np.float32)
    b2 = np.asarray(inputs["b2"], np.float32)

    has_b1 = bool(np.any(b1))
    has_wlb = bool(np.any(wl_b))
    has_b2 = bool(np.any(b2))

    bf = ml_dtypes.bfloat16
    w1b = w1.astype(bf)
    # 0.5x pre-scale compensates the kernel's tanh-based GLU producing 2*xg
    wlw_pad = np.zeros((C, HKP), np.float32)
    wlw_pad[:, :HK] = 0.5 * wl_w
    wlwb = wlw_pad.astype(bf)
    w2b = (0.5 * w2).astype(bf)

    eyeb = np.eye(128).astype(bf)
    in_maps = []
    for c in range(NCORES):
        xs = x[:, c * BPC:(c + 1) * BPC, :]  # (T, BPC, C)
        xt = np.ascontiguousarray(xs.transpose(1, 2, 0)).astype(bf)  # (BPC,C,T)
        m = {"xt": xt, "w1": w1b, "wl_w": wlwb, "w2": w2b, "eye": eyeb}
        if has_b1:
            m["b1"] = b1
        if has_wlb:
            m["wl_b"] = wl_b
        if has_b2:
            m["b2"] = b2
        in_maps.append(m)
    return in_maps


def kernel(x, w1, b1, wl_w, wl_b, w2, b2):
    from concourse.bass_utils import run_bass_kernel_spmd

    b1 = np.asarray(b1, np.float32)
    wl_b = np.asarray(wl_b, np.float32)
    b2 = np.asarray(b2, np.float32)

    has_b1 = bool(np.any(b1))
    has_wlb = bool(np.any(wl_b))
    has_b2 = bool(np.any(b2))

    key = (has_b1, has_wlb, has_b2)
    if key not in _cache:
        _cache[key] = _build(*key)
    nc = _cache[key]

    in_maps = _make_in_maps({
        "x": x, "w1": w1, "b1": b1, "wl_w": wl_w,
        "wl_b": wl_b, "w2": w2, "b2": b2,
    })

    res = run_bass_kernel_spmd(nc, in_maps, core_ids=list(range(NCORES)))
    out = np.empty((T, B, C), np.float32)
    for c in range(NCORES):
        out[:, c * BPC:(c + 1) * BPC, :] = np.asarray(
            res.results[c]["out"]).astype(np.float32)
    return out


# revision 23
# speedup vs baseline: 1.4249x; 1.0431x over previous
"""Trainium2 Bass kernel for nn_CATCallerEncoderLayer (dynamic-conv encoder layer).

Reference computation (T=1024, B=16, C=512, H=8, K=31, P=15):
  h  = x @ w1 + b1; a, g = split(h); xg = a * sigmoid(g)
  w  = softmax((xg @ wl_w + wl_b).reshape(T,B,H,K), axis=-1)
  out[t,b,h*64+r] = sum_k w[t,b,h,k] * xg_pad[t+k-15, b, h*64+r]
  return out @ w2 + b2

Sharding: data-parallel over batch B across 8 cores (2 batches/core). Host
supplies x pre-transposed to feature-major [b, C, T] in bf16 and the weights
in bf16 (wl_w zero-padded to 256 cols). All matmuls run in bf16.

Per core (b = batch slot, 2 per core):
  Phase A per 128-token tile:
    mm1 (bf16, lhsT = xT slices) -> h1a/h1g PSUM -> sigmoid (ACT) ->
    xgb = a*sig(g) bf16 (DVE) -> DRAM xg_pad (halo source)
    xgT via 4 SBUF->SBUF DMA-xbar transposes -> mm_dyn (bf16, N=256)
    exp (ACT) -> group-sum/recip (DVE) -> wsoft bf16
    M-form shear-write (bf16) to zero-filled DRAM staging:
      stage[(b,i,h)-block row t_loc, col t_loc+k] = wsoft[t, h, k]
  Phase B per 98-token conv block:
    band8 [s, (h,t)] via ONE DMA-xbar transpose of the [784, 128] stage block
    halo [128, 512] read from DRAM xg_pad (token rows t0-15 .. t0+113)
    conv psum[c64x2, 4*blk] += halo[:, h].T @ band8[:, h]  (banded matmul)
    convT[b] feature-major accumulator [128, 4, 1024] bf16
  mm2 per 128-token tile (bf16, lhsT = convT slices) -> out bf16 -> DRAM
Host casts the bf16 output back to f32.
"""
import sys

sys.path.insert(0, "/opt/trn_rl_repo")

import numpy as np
import ml_dtypes

T, B, C = 1024, 16, 512
H, KT, PAD = 8, 31, 15
HK = H * KT          # 248
HKP = 256            # wl_w padded cols
NCORES = 8
BPC = B // NCORES    # 2
BT = 98              # conv time-block (s_pad = t + k <= 127)
NBLK = (T + BT - 1) // BT  # 11
NTC = T // 128       # 8
NCC = C // 128       # 4
SBLK = BT * 128      # staging elements per (b,i,h) block = 12544
XGPR = BT * (NBLK - 1) + 128   # xg_pad rows per batch = 1108

_cache = {}
_DEBUG_OUTPUTS = False
_BARRIER_AFTER_ZF = False
# SBUF->SBUF DMA-xbar transposes corrupt concurrent DMA traffic on this HW
# (observed as nondeterministic garbage under multi-core HBM contention), so
# the token->feature xg transpose runs on the PE instead. DRAM->SBUF xbar
# transposes (band8 readback) are unaffected.
_PE_XGT = True


def _split_sync_waits(nc, mybir, max_waits=1):
    """This walrus build rejects instructions carrying >1 sync-wait command.
    Hoist extra waits onto same-engine NOPs inserted just before."""
    cnt = 0
    for f in nc.m.functions:
        for bb in f.blocks:
            new = []
            for inst in bb.instructions:
                si = inst.sync_info
                if si is not None and si.on_wait and len(si.on_wait) > max_waits:
                    waits = list(si.on_wait)
                    for w in waits[:-max_waits]:
                        cnt += 1
                        new.append(
                            mybir.InstNoOp(
                                name=f"I-ws{cnt}",
                                engine=inst.engine,
                                sync_info=mybir.SyncInfo(on_wait=[w], on_update=[]),
                            )
                        )
                    inst.sync_info = mybir.SyncInfo(
                        on_wait=waits[-max_waits:], on_update=list(si.on_update or [])
                    )
                new.append(inst)
            bb.instructions = new
    return cnt


def _build(has_b1, has_wlb, has_b2):
    import bass_rust
    import concourse.bass as bass
    import concourse.tile as tile
    from concourse import mybir

    f32 = mybir.dt.float32
    bf16 = mybir.dt.bfloat16
    AF = mybir.ActivationFunctionType
    AX = mybir.AxisListType

    nc = bass.Bass("TRN2", debug=False)

    xt_d = nc.dram_tensor("xt", (BPC, C, T), bf16, kind="ExternalInput").ap()
    w1_d = nc.dram_tensor("w1", (C, 2 * C), bf16, kind="ExternalInput").ap()
    wlw_d = nc.dram_tensor("wl_w", (C, HKP), bf16, kind="ExternalInput").ap()
    w2_d = nc.dram_tensor("w2", (C, C), bf16, kind="ExternalInput").ap()
    eye_d = nc.dram_tensor("eye", (128, 128), bf16, kind="ExternalInput").ap()
    if has_b1:
        b1_d = nc.dram_tensor("b1", (2 * C,), f32, kind="ExternalInput").ap()
    if has_wlb:
        wlb_d = nc.dram_tensor("wl_b", (HK,), f32, kind="ExternalInput").ap()
    if has_b2:
        b2_d = nc.dram_tensor("b2", (C,), f32, kind="ExternalInput").ap()
    out_d = nc.dram_tensor("out", (T, BPC, C), bf16, kind="ExternalOutput").ap()
    dbg = "ExternalOutput" if _DEBUG_OUTPUTS else "Internal"
    # bf16 M-form band staging: per (b,i,h) block of [98, 128]
    n_srow = BPC * NBLK * H * BT        # 17248 rows
    stage = nc.dram_tensor("stage", (n_srow, 128), bf16, kind=dbg).ap()
    stage_f = stage[:].flatten()
    # zero-padded token-major xg for conv halos: row r = token r-15
    xgp_d = nc.dram_tensor("xgp", (BPC, XGPR, C), bf16, kind=dbg).ap()

    def sbase(b, i, h=0):
        return ((b * NBLK + i) * H + h) * SBLK

    with tile.TileContext(nc) as tc:
        with (
            tc.tile_pool(name="consts", bufs=1) as cpool,
            tc.tile_pool(name="xt", bufs=BPC * NCC) as xtpool,
            tc.tile_pool(name="convT", bufs=BPC) as ctpool,
            tc.tile_pool(name="xgb", bufs=5) as xgbpool,
            tc.tile_pool(name="xgT", bufs=4) as xgtpool,
            tc.tile_pool(name="sg", bufs=4) as sgpool,
            tc.tile_pool(name="soft", bufs=5) as spool,
            tc.tile_pool(name="band", bufs=6) as bpool,
            tc.tile_pool(name="halo", bufs=6) as hpool,
            tc.tile_pool(name="outp", bufs=4) as opool,
            tc.tile_pool(name="psA", bufs=2, space="PSUM") as psA,
            tc.tile_pool(name="psW", bufs=1, space="PSUM") as psW,
            tc.tile_pool(name="psB", bufs=1, space="PSUM") as psB,
            tc.tile_pool(name="psO", bufs=1, space="PSUM") as psO,
            tc.tile_pool(name="psT", bufs=1, space="PSUM") as psT,
        ):
            # ---- constants ----
            w1_t, wlw_t, w2_t = [], [], []
            for cc in range(NCC):
                tw1 = cpool.tile([128, 2 * C], bf16, tag=f"w1_{cc}")
                nc.gpsimd.dma_start(tw1[:], w1_d[cc * 128:(cc + 1) * 128, :])
                w1_t.append(tw1)
                twl = cpool.tile([128, HKP], bf16, tag=f"wlw_{cc}")
                nc.gpsimd.dma_start(twl[:], wlw_d[cc * 128:(cc + 1) * 128, :])
                wlw_t.append(twl)
                tw2 = cpool.tile([128, C], bf16, tag=f"w2_{cc}")
                nc.gpsimd.dma_start(tw2[:], w2_d[cc * 128:(cc + 1) * 128, :])
                w2_t.append(tw2)
            if has_b1:
                b1a = cpool.tile([128, C], f32, tag="b1a")
                nc.gpsimd.dma_start(b1a[:], b1_d[None, 0:C].to_broadcast((128, C)))
                b1g = cpool.tile([128, C], f32, tag="b1g")
                nc.gpsimd.dma_start(b1g[:], b1_d[None, C:2 * C].to_broadcast((128, C)))
            if has_wlb:
                wlb = cpool.tile([128, HK], f32, tag="wlb")
                nc.gpsimd.dma_start(wlb[:], wlb_d[None, :].to_broadcast((128, HK)))
            if has_b2:
                b2t = cpool.tile([128, C], f32, tag="b2t")
                nc.gpsimd.dma_start(b2t[:], b2_d[None, :].to_broadcast((128, C)))

            # zero-fill staging + xg_pad edges (garbage-free band/halo reads)
            zt = cpool.tile([128, 2048], bf16, tag="zt")
            nc.vector.memset(zt[:], 0.0)
            pos = 0
            while pos < n_srow:
                n = min(2048, n_srow - pos)
                nc.scalar.dma_start(stage[pos:pos + n, :], zt[:, :n])
                pos += n
            for b in range(BPC):
                nc.scalar.dma_start(xgp_d[b, 0:PAD, :], zt[:, :PAD * NCC])
                ztail = XGPR - (T + PAD)   # 69
                nc.scalar.dma_start(xgp_d[b, T + PAD:XGPR, :], zt[:, :ztail * NCC])

            xt_t = {}
            for b in range(BPC):
                for cc in range(NCC):
                    tx = xtpool.tile([128, T], bf16, tag="xt")
                    nc.gpsimd.dma_start(tx[:], xt_d[b, cc * 128:(cc + 1) * 128, :])
                    xt_t[(b, cc)] = tx

            if _BARRIER_AFTER_ZF:
                tc.strict_bb_all_engine_barrier()

            eyeb = cpool.tile([128, 128], bf16, tag="eyeb")
            nc.gpsimd.dma_start(eyeb[:], eye_d[:])

            convT = {}
            for b in range(BPC):
                ct = ctpool.tile([128, NCC, T], bf16, tag="convT")
                convT[b] = ct

            def phase_a(b, tch):
                t0 = tch * 128
                h1a = psA.tile([128, C], f32, tag="h1a")
                h1g = psA.tile([128, C], f32, tag="h1g")
                for cc in range(NCC):
                    lhsT = xt_t[(b, cc)][:, t0:t0 + 128]
                    nc.tensor.matmul(h1a[:], lhsT, w1_t[cc][:, 0:C],
                                     start=(cc == 0), stop=(cc == NCC - 1))
                for cc in range(NCC):
                    lhsT = xt_t[(b, cc)][:, t0:t0 + 128]
                    nc.tensor.matmul(h1g[:], lhsT, w1_t[cc][:, C:2 * C],
                                     start=(cc == 0), stop=(cc == NCC - 1))
                # GLU via tanh (keeps ACT on the exp_and_others table set):
                #   2*xg = (1 + tanh((g+b1g)/2)) * (a + b1a)
                # The factor 2 is compensated by host-side 0.5x pre-scaling
                # of wl_w and w2.
                sg = sgpool.tile([128, C], bf16, tag="sg")
                if has_b1:
                    gb = sgpool.tile([128, C], f32, tag="gb")
                    nc.vector.tensor_add(gb[:], h1g[:], b1g[:])
                    nc.scalar.activation(sg[:], gb[:], AF.Tanh, scale=0.5)
                else:
                    nc.scalar.activation(sg[:], h1g[:], AF.Tanh, scale=0.5)
                xgb = xgbpool.tile([128, C], bf16, tag="xgb")
                AL = mybir.AluOpType
                if has_b1:
                    ab = sgpool.tile([128, C], f32, tag="ab")
                    nc.vector.tensor_add(ab[:], h1a[:], b1a[:])
                    nc.vector.scalar_tensor_tensor(
                        xgb[:], sg[:], 1.0, ab[:], AL.add, AL.mult)
                else:
                    nc.vector.scalar_tensor_tensor(
                        xgb[:], sg[:], 1.0, h1a[:], AL.add, AL.mult)
                # halo source in DRAM (row r = token r-15)
                nc.gpsimd.dma_start(xgp_d[b, PAD + t0:PAD + t0 + 128, :], xgb[:])
                # feature-major xg
                xgT = xgtpool.tile([128, C], bf16, tag="xgT")
                if _PE_XGT:
                    tp = psT.tile([128, C], bf16, tag="tp")
                    for j in range(NCC):
                        nc.tensor.transpose(tp[:, j * 128:(j + 1) * 128],
                                            xgb[:, j * 128:(j + 1) * 128],
                                            eyeb[:])
                    nc.vector.tensor_copy(xgT[:], tp[:])
                else:
                    for j in range(NCC):
                        nc.scalar.dma_start_transpose(
                            xgT[:, j * 128:(j + 1) * 128],
                            xgb[:, j * 128:(j + 1) * 128])
                # mm_dyn
                w248 = psW.tile([128, HKP], f32, tag="w248")
                for j in range(NCC):
                    nc.tensor.matmul(w248[:], xgT[:, j * 128:(j + 1) * 128],
                                     wlw_t[j][:],
                                     start=(j == 0), stop=(j == NCC - 1))
                # softmax over taps (values tiny: skip max-subtract)
                we = spool.tile([128, HK], bf16, tag="we")
                if has_wlb:
                    wb = spool.tile([128, HK], f32, tag="wb")
                    nc.vector.tensor_add(wb[:], w248[:, 0:HK], wlb[:])
                    nc.scalar.activation(we[:], wb[:], AF.Exp)
                else:
                    nc.scalar.activation(we[:], w248[:, 0:HK], AF.Exp)
                sums = spool.tile([128, H], f32, tag="sums")
                we3 = we[:].rearrange("t (h k) -> t h k", k=KT)
                nc.vector.reduce_sum(sums[:], we3, axis=AX.X)
                rec = spool.tile([128, H], f32, tag="rec")
                nc.vector.reciprocal(rec[:], sums[:])
                wsb = spool.tile([128, HK], bf16, tag="wsb")
                ws3 = wsb[:].rearrange("t (h k) -> t h k", k=KT)
                rec3 = rec[:, :, None].to_broadcast((128, H, KT))
                nc.vector.tensor_mul(ws3, we3, rec3)
                # M-form shear-write to staging: all 8 heads per DMA,
                # split by overlapped 98-blocks. Element (t,h,k) goes
                # to sbase(b,i,h) + t_loc*129 + k.
                g0, g1 = t0, t0 + 128
                i = g0 // BT
                while i < NBLK and i * BT < g1:
                    r0 = max(g0, i * BT)
                    r1 = min(g1, i * BT + BT)
                    nr = r1 - r0
                    tl0 = r0 - i * BT
                    src = wsb[r0 - g0:r1 - g0, :].rearrange(
                        "t (h k) -> t h k", k=KT)
                    dst = stage_f[:1].copy()
                    dst.ap = bass_rust.VecI64Pair(
                        [[129, nr], [SBLK, H], [1, KT]])
                    dst.offset = sbase(b, i) + tl0 * 129
                    nc.sync.dma_start(dst, src)
                    i += 1

            def conv_block(b, i):
                t0 = i * BT
                blk = min(BT, T - t0)
                # band8[s, h*98+t] via one DMA-xbar transpose of stage block
                band8 = bpool.tile([128, H * BT], bf16, tag="band8")
                r0 = (b * NBLK + i) * H * BT
                nc.scalar.dma_start_transpose(band8[:], stage[r0:r0 + H * BT, :])
                # halo: token rows [t0-15, t0+113) = xgp rows [t0, t0+128)
                halo = hpool.tile([128, C], bf16, tag="halo")
                nc.gpsimd.dma_start(halo[:], xgp_d[b, t0:t0 + 128, :])
                # banded conv matmuls: 2 heads per psum tile via col groups
                cp = psB.tile([128, NCC * blk], f32, tag="cp")
                for j in range(NCC):
                    for jj in range(2):
                        h = 2 * j + jj
                        nc.tensor.matmul(
                            cp[jj * 64:(jj + 1) * 64, j * blk:(j + 1) * blk],
                            halo[:, h * 64:(h + 1) * 64],
                            band8[:, h * BT:h * BT + blk],
                            start=True, stop=True,
                            tile_position=(0, 64 * jj),
                        )
                # feature-major conv accumulator (bf16) for mm2 lhsT
                cp3 = cp[:].rearrange("p (j t) -> p j t", j=NCC)
                nc.vector.tensor_copy(convT[b][:, :, t0:t0 + blk], cp3)

            def mm2_tile(b, tch):
                t0 = tch * 128
                op = psO.tile([128, C], f32, tag="op")
                for j in range(NCC):
                    nc.tensor.matmul(op[:], convT[b][:, j, t0:t0 + 128],
                                     w2_t[j][:],
                                     start=(j == 0), stop=(j == NCC - 1))
                os = opool.tile([128, C], bf16, tag="os")
                if has_b2:
                    nc.vector.tensor_add(os[:], op[:], b2t[:])
                else:
                    nc.scalar.copy(os[:], op[:])
                nc.sync.dma_start(out_d[t0:t0 + 128, b, :], os[:])

            # interleave the two batch slots: two independent dependency
            # chains keep the PE busy while ACT/DVE/DMA work on the other
            for tch in range(NTC):
                for b in range(BPC):
                    phase_a(b, tch)
            mm2_done = [0] * BPC
            for i in range(NBLK):
                for b in range(BPC):
                    conv_block(b, i)
                for b in range(BPC):
                    # interleave mm2 for fully-covered token tiles
                    while (mm2_done[b] + 1) * 128 <= (i + 1) * BT \
                            and mm2_done[b] < NTC:
                        mm2_tile(b, mm2_done[b])
                        mm2_done[b] += 1
            for b in range(BPC):
                while mm2_done[b] < NTC:
                    mm2_tile(b, mm2_done[b])
                    mm2_done[b] += 1

    _split_sync_waits(nc, mybir)
    return nc


def _make_in_maps(inputs):
    x = np.asarray(inputs["x"], np.float32)
    w1 = np.asarray(inputs["w1"], np.float32)
    b1 = np.asarray(inputs["b1"], np.float32)
    wl_w = np.asarray(inputs["wl_w"], np.float32)
    wl_b = np.asarray(inputs["wl_b"], np.float32)
    w2 = np.asarray(inputs["w2"], np.float32)
    b2 = np.asarray(inputs["b2"], np.float32)

    has_b1 = bool(np.any(b1))
    has_wlb = bool(np.any(wl_b))
    has_b2 = bool(np.any(b2))

    bf = ml_dtypes.bfloat16
    w1b = w1.astype(bf)
    # 0.5x pre-scale compensates the kernel's tanh-based GLU producing 2*xg
    wlw_pad = np.zeros((C, HKP), np.float32)
    wlw_pad[:, :HK] = 0.5 * wl_w
    wlwb = wlw_pad.astype(bf)
    w2b = (0.5 * w2).astype(bf)

    eyeb = np.eye(128).astype(bf)
    in_maps = []
    for c in range(NCORES):
        xs = x[:, c * BPC:(c + 1) * BPC, :]  # (T, BPC, C)
        xt = np.ascontiguousarray(xs.transpose(1, 2, 0)).astype(bf)  # (BPC,C,T)
        m = {"xt": xt, "w1": w1b, "wl_w": wlwb, "w2": w2b, "eye": eyeb}
        if has_b1:
            m["b1"] = b1
        if has_wlb:
            m["wl_b"] = wl_b
        if has_b2:
            m["b2"] = b2
        in_maps.append(m)
    return in_maps


def kernel(x, w1, b1, wl_w, wl_b, w2, b2):
    from concourse.bass_utils import run_bass_kernel_spmd

    b1 = np.asarray(b1, np.float32)
    wl_b = np.asarray(wl_b, np.float32)
    b2 = np.asarray(b2, np.float32)

    has_b1 = bool(np.any(b1))
    has_wlb = bool(np.any(wl_b))
    has_b2 = bool(np.any(b2))

    key = (has_b1, has_wlb, has_b2)
    if key not in _cache:
        _cache[key] = _build(*key)
    nc = _cache[key]

    in_maps = _make_in_maps({
        "x": x, "w1": w1, "b1": b1, "wl_w": wl_w,
        "wl_b": wl_b, "w2": w2, "b2": b2,
    })

    res = run_bass_kernel_spmd(nc, in_maps, core_ids=list(range(NCORES)))
    out = np.empty((T, B, C), np.float32)
    for c in range(NCORES):
        out[:, c * BPC:(c + 1) * BPC, :] = np.asarray(
            res.results[c]["out"]).astype(np.float32)
    return out


# revision 26
# speedup vs baseline: 1.5148x; 1.0631x over previous
"""Trainium2 Bass kernel for nn_CATCallerEncoderLayer (dynamic-conv encoder layer).

Reference computation (T=1024, B=16, C=512, H=8, K=31, P=15):
  h  = x @ w1 + b1; a, g = split(h); xg = a * sigmoid(g)
  w  = softmax((xg @ wl_w + wl_b).reshape(T,B,H,K), axis=-1)
  out[t,b,h*64+r] = sum_k w[t,b,h,k] * xg_pad[t+k-15, b, h*64+r]
  return out @ w2 + b2

Sharding: data-parallel over batch B across 8 cores (2 batches/core). Host
supplies x pre-transposed to feature-major [b, C, T] in bf16 and the weights
in bf16 (wl_w zero-padded to 256 cols). All matmuls run in bf16.

Per core (b = batch slot, 2 per core):
  Phase A per 128-token tile:
    mm1 (bf16, lhsT = xT slices) -> h1a/h1g PSUM -> sigmoid (ACT) ->
    xgb = a*sig(g) bf16 (DVE) -> DRAM xg_pad (halo source)
    xgT via 4 SBUF->SBUF DMA-xbar transposes -> mm_dyn (bf16, N=256)
    exp (ACT) -> group-sum/recip (DVE) -> wsoft bf16
    M-form shear-write (bf16) to zero-filled DRAM staging:
      stage[(b,i,h)-block row t_loc, col t_loc+k] = wsoft[t, h, k]
  Phase B per 98-token conv block:
    band8 [s, (h,t)] via ONE DMA-xbar transpose of the [784, 128] stage block
    halo [128, 512] read from DRAM xg_pad (token rows t0-15 .. t0+113)
    conv psum[c64x2, 4*blk] += halo[:, h].T @ band8[:, h]  (banded matmul)
    convT[b] feature-major accumulator [128, 4, 1024] bf16
  mm2 per 128-token tile (bf16, lhsT = convT slices) -> out bf16 -> DRAM
Host casts the bf16 output back to f32.
"""
import sys

sys.path.insert(0, "/opt/trn_rl_repo")

import numpy as np
import ml_dtypes

T, B, C = 1024, 16, 512
H, KT, PAD = 8, 31, 15
HK = H * KT          # 248
HKP = 256            # wl_w padded cols
NCORES = 8
BPC = B // NCORES    # 2
BT = 98              # conv time-block (s_pad = t + k <= 127)
NBLK = (T + BT - 1) // BT  # 11
NTC = T // 128       # 8
NCC = C // 128       # 4
SBLK = BT * 128      # staging elements per (b,i,h) block = 12544
XGPR = BT * (NBLK - 1) + 128   # xg_pad rows per batch = 1108

_cache = {}
_DEBUG_OUTPUTS = False
_BARRIER_AFTER_ZF = False
# SBUF->SBUF DMA-xbar transposes corrupt concurrent DMA traffic on this HW
# (observed as nondeterministic garbage under multi-core HBM contention), so
# the token->feature xg transpose runs on the PE instead. DRAM->SBUF xbar
# transposes (band8 readback) are unaffected.
_PE_XGT = True


def _split_sync_waits(nc, mybir, max_waits=1):
    """This walrus build rejects instructions carrying >1 sync-wait command.
    Hoist extra waits onto same-engine NOPs inserted just before."""
    cnt = 0
    for f in nc.m.functions:
        for bb in f.blocks:
            new = []
            for inst in bb.instructions:
                si = inst.sync_info
                if si is not None and si.on_wait and len(si.on_wait) > max_waits:
                    waits = list(si.on_wait)
                    for w in waits[:-max_waits]:
                        cnt += 1
                        new.append(
                            mybir.InstNoOp(
                                name=f"I-ws{cnt}",
                                engine=inst.engine,
                                sync_info=mybir.SyncInfo(on_wait=[w], on_update=[]),
                            )
                        )
                    inst.sync_info = mybir.SyncInfo(
                        on_wait=waits[-max_waits:], on_update=list(si.on_update or [])
                    )
                new.append(inst)
            bb.instructions = new
    return cnt


def _build(has_b1, has_wlb, has_b2):
    import bass_rust
    import concourse.bass as bass
    import concourse.tile as tile
    from concourse import mybir

    f32 = mybir.dt.float32
    bf16 = mybir.dt.bfloat16
    AF = mybir.ActivationFunctionType
    AX = mybir.AxisListType

    nc = bass.Bass("TRN2", debug=False)

    xt_d = nc.dram_tensor("xt", (BPC, C, T), bf16, kind="ExternalInput").ap()
    w1_d = nc.dram_tensor("w1", (C, 2 * C), bf16, kind="ExternalInput").ap()
    wlw_d = nc.dram_tensor("wl_w", (C, HKP), bf16, kind="ExternalInput").ap()
    w2_d = nc.dram_tensor("w2", (C, C), bf16, kind="ExternalInput").ap()
    eye_d = nc.dram_tensor("eye", (128, 128), bf16, kind="ExternalInput").ap()
    if has_b1:
        b1_d = nc.dram_tensor("b1", (2 * C,), f32, kind="ExternalInput").ap()
    if has_wlb:
        wlb_d = nc.dram_tensor("wl_b", (HK,), f32, kind="ExternalInput").ap()
    if has_b2:
        b2_d = nc.dram_tensor("b2", (C,), f32, kind="ExternalInput").ap()
    out_d = nc.dram_tensor("out", (T, BPC, C), bf16, kind="ExternalOutput").ap()
    dbg = "ExternalOutput" if _DEBUG_OUTPUTS else "Internal"
    # bf16 M-form band staging: per (b,i,h) block of [98, 128]
    n_srow = BPC * NBLK * H * BT        # 17248 rows
    stage = nc.dram_tensor("stage", (n_srow, 128), bf16, kind=dbg).ap()
    stage_f = stage[:].flatten()
    # zero-padded token-major xg for conv halos: row r = token r-15
    xgp_d = nc.dram_tensor("xgp", (BPC, XGPR, C), bf16, kind=dbg).ap()

    def sbase(b, i, h=0):
        return ((b * NBLK + i) * H + h) * SBLK

    with tile.TileContext(nc) as tc:
        with (
            tc.tile_pool(name="consts", bufs=1) as cpool,
            tc.tile_pool(name="xt", bufs=BPC * NCC) as xtpool,
            tc.tile_pool(name="convT", bufs=BPC) as ctpool,
            tc.tile_pool(name="xgb", bufs=5) as xgbpool,
            tc.tile_pool(name="xgT", bufs=4) as xgtpool,
            tc.tile_pool(name="sg", bufs=4) as sgpool,
            tc.tile_pool(name="soft", bufs=5) as spool,
            tc.tile_pool(name="band", bufs=6) as bpool,
            tc.tile_pool(name="halo", bufs=6) as hpool,
            tc.tile_pool(name="outp", bufs=4) as opool,
            tc.tile_pool(name="psA", bufs=2, space="PSUM") as psA,
            tc.tile_pool(name="psW", bufs=1, space="PSUM") as psW,
            tc.tile_pool(name="psB", bufs=1, space="PSUM") as psB,
            tc.tile_pool(name="psO", bufs=1, space="PSUM") as psO,
            tc.tile_pool(name="psT", bufs=1, space="PSUM") as psT,
        ):
            # ---- constants ----  (sync=HWDGE: fast first-byte; PE waits on these)
            w1_t, wlw_t, w2_t = [], [], []
            for cc in range(NCC):
                tw1 = cpool.tile([128, 2 * C], bf16, tag=f"w1_{cc}")
                nc.sync.dma_start(tw1[:], w1_d[cc * 128:(cc + 1) * 128, :])
                w1_t.append(tw1)
            if has_b1:
                b1a = cpool.tile([128, C], f32, tag="b1a")
                nc.gpsimd.dma_start(b1a[:], b1_d[None, 0:C].to_broadcast((128, C)))
                b1g = cpool.tile([128, C], f32, tag="b1g")
                nc.gpsimd.dma_start(b1g[:], b1_d[None, C:2 * C].to_broadcast((128, C)))
            if has_wlb:
                wlb = cpool.tile([128, HK], f32, tag="wlb")
                nc.gpsimd.dma_start(wlb[:], wlb_d[None, :].to_broadcast((128, HK)))
            if has_b2:
                b2t = cpool.tile([128, C], f32, tag="b2t")
                nc.gpsimd.dma_start(b2t[:], b2_d[None, :].to_broadcast((128, C)))

            # zero-fill staging + xg_pad edges (garbage-free band/halo reads)
            zt = cpool.tile([128, 2048], bf16, tag="zt")
            nc.vector.memset(zt[:], 0.0)
            pos = 0
            while pos < n_srow:
                n = min(2048, n_srow - pos)
                nc.gpsimd.dma_start(stage[pos:pos + n, :], zt[:, :n])
                pos += n
            for b in range(BPC):
                nc.gpsimd.dma_start(xgp_d[b, 0:PAD, :], zt[:, :PAD * NCC])
                ztail = XGPR - (T + PAD)   # 69
                nc.gpsimd.dma_start(xgp_d[b, T + PAD:XGPR, :], zt[:, :ztail * NCC])

            xt_t = {}
            for b in range(BPC):
                for cc in range(NCC):
                    tx = xtpool.tile([128, T], bf16, tag="xt")
                    nc.sync.dma_start(tx[:], xt_d[b, cc * 128:(cc + 1) * 128, :])
                    xt_t[(b, cc)] = tx
            eyeb = cpool.tile([128, 128], bf16, tag="eyeb")
            nc.sync.dma_start(eyeb[:], eye_d[:])
            for cc in range(NCC):
                twl = cpool.tile([128, HKP], bf16, tag=f"wlw_{cc}")
                nc.sync.dma_start(twl[:], wlw_d[cc * 128:(cc + 1) * 128, :])
                wlw_t.append(twl)
                tw2 = cpool.tile([128, C], bf16, tag=f"w2_{cc}")
                nc.sync.dma_start(tw2[:], w2_d[cc * 128:(cc + 1) * 128, :])
                w2_t.append(tw2)

            if _BARRIER_AFTER_ZF:
                tc.strict_bb_all_engine_barrier()

            convT = {}
            for b in range(BPC):
                ct = ctpool.tile([128, NCC, T], bf16, tag="convT")
                convT[b] = ct

            def phase_a(b, tch):
                t0 = tch * 128
                h1a = psA.tile([128, C], f32, tag="h1a")
                h1g = psA.tile([128, C], f32, tag="h1g")
                for cc in range(NCC):
                    lhsT = xt_t[(b, cc)][:, t0:t0 + 128]
                    nc.tensor.matmul(h1a[:], lhsT, w1_t[cc][:, 0:C],
                                     start=(cc == 0), stop=(cc == NCC - 1))
                for cc in range(NCC):
                    lhsT = xt_t[(b, cc)][:, t0:t0 + 128]
                    nc.tensor.matmul(h1g[:], lhsT, w1_t[cc][:, C:2 * C],
                                     start=(cc == 0), stop=(cc == NCC - 1))
                # GLU via tanh (keeps ACT on the exp_and_others table set):
                #   2*xg = (1 + tanh((g+b1g)/2)) * (a + b1a)
                # The factor 2 is compensated by host-side 0.5x pre-scaling
                # of wl_w and w2.
                sg = sgpool.tile([128, C], bf16, tag="sg")
                if has_b1:
                    gb = sgpool.tile([128, C], f32, tag="gb")
                    nc.vector.tensor_add(gb[:], h1g[:], b1g[:])
                    nc.scalar.activation(sg[:], gb[:], AF.Tanh, scale=0.5)
                else:
                    nc.scalar.activation(sg[:], h1g[:], AF.Tanh, scale=0.5)
                xgb = xgbpool.tile([128, C], bf16, tag="xgb")
                AL = mybir.AluOpType
                if has_b1:
                    ab = sgpool.tile([128, C], f32, tag="ab")
                    nc.vector.tensor_add(ab[:], h1a[:], b1a[:])
                    nc.vector.scalar_tensor_tensor(
                        xgb[:], sg[:], 1.0, ab[:], AL.add, AL.mult)
                else:
                    nc.vector.scalar_tensor_tensor(
                        xgb[:], sg[:], 1.0, h1a[:], AL.add, AL.mult)
                # halo source in DRAM (row r = token r-15)
                nc.gpsimd.dma_start(xgp_d[b, PAD + t0:PAD + t0 + 128, :], xgb[:])
                # feature-major xg
                xgT = xgtpool.tile([128, C], bf16, tag="xgT")
                if _PE_XGT:
                    tp = psT.tile([128, C], bf16, tag="tp")
                    for j in range(NCC):
                        nc.tensor.transpose(tp[:, j * 128:(j + 1) * 128],
                                            xgb[:, j * 128:(j + 1) * 128],
                                            eyeb[:])
                    nc.vector.tensor_copy(xgT[:], tp[:])
                else:
                    for j in range(NCC):
                        nc.scalar.dma_start_transpose(
                            xgT[:, j * 128:(j + 1) * 128],
                            xgb[:, j * 128:(j + 1) * 128])
                # mm_dyn
                w248 = psW.tile([128, HKP], f32, tag="w248")
                for j in range(NCC):
                    nc.tensor.matmul(w248[:], xgT[:, j * 128:(j + 1) * 128],
                                     wlw_t[j][:],
                                     start=(j == 0), stop=(j == NCC - 1))
                # softmax over taps (values tiny: skip max-subtract)
                we = spool.tile([128, HK], bf16, tag="we")
                if has_wlb:
                    wb = spool.tile([128, HK], f32, tag="wb")
                    nc.vector.tensor_add(wb[:], w248[:, 0:HK], wlb[:])
                    nc.scalar.activation(we[:], wb[:], AF.Exp)
                else:
                    nc.scalar.activation(we[:], w248[:, 0:HK], AF.Exp)
                sums = spool.tile([128, H], f32, tag="sums")
                we3 = we[:].rearrange("t (h k) -> t h k", k=KT)
                nc.vector.reduce_sum(sums[:], we3, axis=AX.X)
                rec = spool.tile([128, H], f32, tag="rec")
                nc.vector.reciprocal(rec[:], sums[:])
                wsb = spool.tile([128, HK], bf16, tag="wsb")
                ws3 = wsb[:].rearrange("t (h k) -> t h k", k=KT)
                rec3 = rec[:, :, None].to_broadcast((128, H, KT))
                nc.vector.tensor_mul(ws3, we3, rec3)
                # M-form shear-write to staging: all 8 heads per DMA,
                # split by overlapped 98-blocks. Element (t,h,k) goes
                # to sbase(b,i,h) + t_loc*129 + k.
                g0, g1 = t0, t0 + 128
                i = g0 // BT
                while i < NBLK and i * BT < g1:
                    r0 = max(g0, i * BT)
                    r1 = min(g1, i * BT + BT)
                    nr = r1 - r0
                    tl0 = r0 - i * BT
                    src = wsb[r0 - g0:r1 - g0, :].rearrange(
                        "t (h k) -> t h k", k=KT)
                    dst = stage_f[:1].copy()
                    dst.ap = bass_rust.VecI64Pair(
                        [[129, nr], [SBLK, H], [1, KT]])
                    dst.offset = sbase(b, i) + tl0 * 129
                    nc.sync.dma_start(dst, src)
                    i += 1

            def conv_block(b, i):
                t0 = i * BT
                blk = min(BT, T - t0)
                # band8[s, h*98+t] via one DMA-xbar transpose of stage block
                band8 = bpool.tile([128, H * BT], bf16, tag="band8")
                r0 = (b * NBLK + i) * H * BT
                nc.scalar.dma_start_transpose(band8[:], stage[r0:r0 + H * BT, :])
                # halo: token rows [t0-15, t0+113) = xgp rows [t0, t0+128)
                halo = hpool.tile([128, C], bf16, tag="halo")
                nc.gpsimd.dma_start(halo[:], xgp_d[b, t0:t0 + 128, :])
                # banded conv matmuls: 2 heads per psum tile via col groups
                cp = psB.tile([128, NCC * blk], f32, tag="cp")
                for j in range(NCC):
                    for jj in range(2):
                        h = 2 * j + jj
                        nc.tensor.matmul(
                            cp[jj * 64:(jj + 1) * 64, j * blk:(j + 1) * blk],
                            halo[:, h * 64:(h + 1) * 64],
                            band8[:, h * BT:h * BT + blk],
                            start=True, stop=True,
                            tile_position=(0, 64 * jj),
                        )
                # feature-major conv accumulator (bf16) for mm2 lhsT
                cp3 = cp[:].rearrange("p (j t) -> p j t", j=NCC)
                nc.vector.tensor_copy(convT[b][:, :, t0:t0 + blk], cp3)

            def mm2_tile(b, tch):
                t0 = tch * 128
                op = psO.tile([128, C], f32, tag="op")
                for j in range(NCC):
                    nc.tensor.matmul(op[:], convT[b][:, j, t0:t0 + 128],
                                     w2_t[j][:],
                                     start=(j == 0), stop=(j == NCC - 1))
                os = opool.tile([128, C], bf16, tag="os")
                if has_b2:
                    nc.vector.tensor_add(os[:], op[:], b2t[:])
                else:
                    nc.scalar.copy(os[:], op[:])
                nc.sync.dma_start(out_d[t0:t0 + 128, b, :], os[:])

            # interleave the two batch slots: two independent dependency
            # chains keep the PE busy while ACT/DVE/DMA work on the other
            for tch in range(NTC):
                for b in range(BPC):
                    phase_a(b, tch)
            mm2_done = [0] * BPC
            for i in range(NBLK):
                for b in range(BPC):
                    conv_block(b, i)
                for b in range(BPC):
                    # interleave mm2 for fully-covered token tiles
                    while (mm2_done[b] + 1) * 128 <= (i + 1) * BT \
                            and mm2_done[b] < NTC:
                        mm2_tile(b, mm2_done[b])
                        mm2_done[b] += 1
            for b in range(BPC):
                while mm2_done[b] < NTC:
                    mm2_tile(b, mm2_done[b])
                    mm2_done[b] += 1

    _split_sync_waits(nc, mybir)
    return nc


def _make_in_maps(inputs):
    x = np.asarray(inputs["x"], np.float32)
    w1 = np.asarray(inputs["w1"], np.float32)
    b1 = np.asarray(inputs["b1"], np.float32)
    wl_w = np.asarray(inputs["wl_w"], np.float32)
    wl_b = np.asarray(inputs["wl_b"], np.float32)
    w2 = np.asarray(inputs["w2"], np.float32)
    b2 = np.asarray(inputs["b2"], np.float32)

    has_b1 = bool(np.any(b1))
    has_wlb = bool(np.any(wl_b))
    has_b2 = bool(np.any(b2))

    bf = ml_dtypes.bfloat16
    w1b = w1.astype(bf)
    # 0.5x pre-scale compensates the kernel's tanh-based GLU producing 2*xg
    wlw_pad = np.zeros((C, HKP), np.float32)
    wlw_pad[:, :HK] = 0.5 * wl_w
    wlwb = wlw_pad.astype(bf)
    w2b = (0.5 * w2).astype(bf)

    eyeb = np.eye(128).astype(bf)
    in_maps = []
    for c in range(NCORES):
        xs = x[:, c * BPC:(c + 1) * BPC, :]  # (T, BPC, C)
        xt = np.ascontiguousarray(xs.transpose(1, 2, 0)).astype(bf)  # (BPC,C,T)
        m = {"xt": xt, "w1": w1b, "wl_w": wlwb, "w2": w2b, "eye": eyeb}
        if has_b1:
            m["b1"] = b1
        if has_wlb:
            m["wl_b"] = wl_b
        if has_b2:
            m["b2"] = b2
        in_maps.append(m)
    return in_maps


def kernel(x, w1, b1, wl_w, wl_b, w2, b2):
    from concourse.bass_utils import run_bass_kernel_spmd

    b1 = np.asarray(b1, np.float32)
    wl_b = np.asarray(wl_b, np.float32)
    b2 = np.asarray(b2, np.float32)

    has_b1 = bool(np.any(b1))
    has_wlb = bool(np.any(wl_b))
    has_b2 = bool(np.any(b2))

    key = (has_b1, has_wlb, has_b2)
    if key not in _cache:
        _cache[key] = _build(*key)
    nc = _cache[key]

    in_maps = _make_in_maps({
        "x": x, "w1": w1, "b1": b1, "wl_w": wl_w,
        "wl_b": wl_b, "w2": w2, "b2": b2,
    })

    res = run_bass_kernel_spmd(nc, in_maps, core_ids=list(range(NCORES)))
    out = np.empty((T, B, C), np.float32)
    for c in range(NCORES):
        out[:, c * BPC:(c + 1) * BPC, :] = np.asarray(
            res.results[c]["out"]).astype(np.float32)
    return out
